# revision 27
# baseline (speedup 1.0000x reference)
"""AdaConv2d fused kernel for 8 TRN2 NeuronCores (pure data parallel).

Per-sample pipeline (all fused on-chip):
  1. instance-norm stats (mean/var over HW)
  2. dynamic per-(b,c) depthwise 3x3 conv with reflect padding
  3. per-(b,c) scale+bias (folded algebraically into the depthwise taps:
     y = A*(sum_t w_t * x_t) + B with A = rstd*w_pt, B = bias - mu*A*sum(w))
  4. fixed 3x3 conv (256->256) with reflect padding, as 18 accumulated
     bf16 matmuls per PSUM bank

Layout: channels on partitions (2 tiles of 128), pixels on the free axis.
Padded images are 66 rows x 66 cols stored flat with a 2-element leading
margin (so every depthwise tap and every matmul rhs is a fully CONTIGUOUS
1D slice).  flat(r, c) = 2 + 66*r + c.  Rows 0/65 and cols 0/65 are the
reflect pads.  A one-element-left-shifted copy (xpb2[i] = xpb[i+1]) keeps
all odd-offset depthwise taps 4-byte aligned for the DVE bf16 2x mode.

v2 scheduling (engine budget per core: PE 263us, DVE ~200us, ACT ~205us):
  - batch 0 tiles use quarter-chunked x DMAs + stats so the pipeline
    latency to the first PE matmul shrinks; sum(x^2) for b0 tiles runs on
    DVE (scalar_tensor_tensor w/ accum) because DVE is idle at startup.
  - dummy fp32 matmuls paced by the b0 DMA chunk arrivals keep the PE HAM
    clock-gate warm so the b0ct0 depthwise-on-PE runs at 2.4 GHz.
  - b0ct0 depthwise runs on the PE (diagonal matmuls) so the PE is not
    gated on DVE at startup; its PSUM drains go to ACT (identity+bias).
  - per tile, 2 of the 8 non-center tap products run on ACT; DVE does
    6 products + 7 tree adds (DVE/batch ~49us vs PE/batch 61us).
  - big conv: ot -> ct -> bank(r0) -> tap order for every batch: 8 PSUM
    banks held, 72-MM runway before ct1 is needed, banks drain (ACT) and
    DMA out per 8-row block as soon as their ct1 taps finish.
"""

import os
from contextlib import ExitStack

import numpy as np

B_GLOBAL = 32
N_CORES = 8
NB = B_GLOBAL // N_CORES  # batches per core
C = 256
H = W = 64
WPAD = W + 2        # 66 padded row length
HPAD = H + 2        # 66 padded rows
MARG = 2            # leading margin so tap windows stay in-bounds
FLAT = MARG + HPAD * WPAD + 2   # 4360 flat elements per padded image
NPIX = H * W        # 4096
CT = C // 128       # channel tiles
OT = C // 128       # out-channel tiles
EPS = 1e-5
BLK_ROWS = 8        # output rows per PSUM bank (8*64=512 fp32, 3D-AP rhs)

ROW_BLOCKS = [(r0, BLK_ROWS) for r0 in range(0, H, BLK_ROWS)]

_CACHED = {}


def _build(nb=NB):
    import concourse.mybir as mybir
    import concourse.tile as tile
    from concourse import bacc

    f32 = mybir.dt.float32
    bf16 = mybir.dt.bfloat16
    AF = mybir.ActivationFunctionType
    ALU = mybir.AluOpType

    nc = bacc.Bacc(None, target_bir_lowering=False)

    x_ext = nc.declare_dram_parameter("x", [nb, C, H, W], f32, isOutput=False)
    wsp_ext = nc.declare_dram_parameter("wsp", [nb, CT, 128, 9], f32, isOutput=False)
    wpt_ext = nc.declare_dram_parameter("wpt", [nb, CT, 128], f32, isOutput=False)
    bis_ext = nc.declare_dram_parameter("bis", [nb, CT, 128], f32, isOutput=False)
    cw_ext = nc.declare_dram_parameter("cw", [CT, 128, 3, 3, OT, 128], bf16, isOutput=False)
    cb_ext = nc.declare_dram_parameter("cb", [OT, 128], f32, isOutput=False)
    out_ext = nc.declare_dram_parameter("out", [nb, C, H, W], f32, isOutput=True)

    with tile.TileContext(nc) as tc, ExitStack() as ctx:
        singles = ctx.enter_context(tc.tile_pool(name="singles", bufs=1))
        xin_pool = ctx.enter_context(tc.tile_pool(name="xin", bufs=2))
        xpb_pool = ctx.enter_context(tc.tile_pool(name="xpb", bufs=2))
        xpb2_pool = ctx.enter_context(tc.tile_pool(name="xpb2", bufs=2))
        yp_pool = ctx.enter_context(tc.tile_pool(name="yp", bufs=4))
        stage_pool = ctx.enter_context(tc.tile_pool(name="stage", bufs=2))
        small_pool = ctx.enter_context(tc.tile_pool(name="small", bufs=4))
        tmp_pool = ctx.enter_context(tc.tile_pool(name="tmp", bufs=2))
        psum_pool = ctx.enter_context(tc.tile_pool(name="psum", bufs=8, space="PSUM"))

        # ---- constants / fixed weights ----
        # (emitted lazily AFTER batch 0's x DMAs so the 1.2MB weight load
        # doesn't delay the first tile's data; weights are only needed at
        # ~30us when the first big-conv matmul fires)
        cw_sb = []

        cb_sb = singles.tile([128, OT], f32, tag="cb")

        def load_cw():
            for ct in range(CT):
                t = singles.tile([128, 3, 3, OT, 128], bf16, tag=f"cw{ct}")
                nc.sync.dma_start(out=t[:], in_=cw_ext[ct])
                cw_sb.append(t)
            for ot in range(OT):
                nc.sync.dma_start(out=cb_sb[:, ot : ot + 1], in_=cb_ext[ot, :, None])

        eps_sb = singles.tile([128, 1], f32, tag="eps")
        nc.vector.memset(eps_sb[:], EPS)
        ident = singles.tile([128, 128], bf16, tag="ident")
        from concourse.masks import make_identity
        make_identity(nc, ident[:])
        # junk fp32 lhsT for the HAM warm-up matmuls
        junkw = singles.tile([128, 128], f32, tag="junkw")
        nc.vector.memset(junkw[:], 0.5)
        # touch the Sqrt activation table once so its ~1.3us load happens
        # before the first tile's stats need it
        warm = singles.tile([128, 1], f32, tag="warm")
        nc.scalar.activation(out=warm[:], in_=eps_sb[:], func=AF.Sqrt, bias=eps_sb[:])

        yp_tiles = {}

        def grid(flat_ap):
            """(128, FLAT) flat padded buffer -> (128, 66, 66) image view."""
            return flat_ap[:, MARG : MARG + HPAD * WPAD].rearrange(
                "p (r c) -> p r c", c=WPAD)

        def fill_borders_dve(buf):
            g = grid(buf[:])
            nc.vector.tensor_scalar_mul(g[:, 1 : 1 + H, 0:1], g[:, 1 : 1 + H, 2:3], 1.0)
            nc.vector.tensor_scalar_mul(g[:, 1 : 1 + H, 65:66], g[:, 1 : 1 + H, 63:64], 1.0)
            nc.vector.tensor_scalar_mul(g[:, 0], g[:, 2], 1.0)
            nc.vector.tensor_scalar_mul(g[:, HPAD - 1], g[:, HPAD - 3], 1.0)

        def warm_mm(rhs_f32, n, label, count=1):
            """Dummy fp32 matmuls that keep the PE HAM clock-gate warm
            while real work is still in flight. Results are never read."""
            ps = psum_pool.tile([128, 512], f32, tag="ps", name=f"psw_{label}")
            for i in range(count):
                nc.tensor.matmul(ps[:, :n], junkw[:], rhs_f32, start=True, stop=True)

        def produce_yp(b, ct, xf_pre=None):
            """norm + depthwise pipeline for one (batch, channel-tile)."""
            first_batch = b == 0
            on_pe = first_batch and ct == 0
            nch = 4 if first_batch else 2       # x DMA / stats chunks
            rows_c = H // nch

            if xf_pre is not None:
                xf = xf_pre
            else:
                xf = xin_pool.tile([128, H, W], f32, tag="xf")
            # fp32 dummies run at HALF rate (427ns warm / 853 cold per
            # N=512) so keep the warm-up lean: enough sustained busy to
            # trip the HAM SHORT window (~3.4us), then stop.
            warm_counts = {0: 3, 1: 2, 2: 1, 3: 1}
            for q in range(nch):
                if xf_pre is None:
                    nc.sync.dma_start(
                        out=xf[:, q * rows_c : (q + 1) * rows_c],
                        in_=x_ext[b, ct * 128 : (ct + 1) * 128, q * rows_c : (q + 1) * rows_c],
                    )
                if on_pe:
                    warm_mm(
                        xf[:, q * rows_c : (q + 1) * rows_c]
                        .rearrange("p a b -> p (a b)")[:, :512],
                        512, f"{b}_{ct}_{q}", warm_counts[q],
                    )

            wsp = small_pool.tile([128, 9], f32, tag="wsp")
            nc.sync.dma_start(out=wsp[:], in_=wsp_ext[b, ct])
            wpt = small_pool.tile([128, 1], f32, tag="wpt")
            nc.sync.dma_start(out=wpt[:], in_=wpt_ext[b, ct, :, None])
            bis = small_pool.tile([128, 1], f32, tag="bis")
            nc.sync.dma_start(out=bis[:], in_=bis_ext[b, ct, :, None])

            xpb = xpb_pool.tile([128, FLAT], bf16, tag="xpb")
            xpb2 = xpb2_pool.tile([128, FLAT], bf16, tag="xpb2")

            # stats: sum(x) fused into the f32->bf16 convert on ACT.
            # sum(x^2): batch-0 tiles compute it on DVE (idle at startup)
            # via scalar_tensor_tensor w/ accum; later tiles use ACT Square.
            # Square scratch: b0ct0 -> yp buffer (overwritten by PE-dw
            # drains later); other tiles -> xpb2 (overwritten by the shift).
            sumx = small_pool.tile([128, nch], f32, tag="sumx")
            sumsq = small_pool.tile([128, nch], f32, tag="sumsq")
            xff = xf[:].rearrange("p a b -> p (a b)")
            yp = yp_pool.tile([128, FLAT], bf16, tag="yp")
            sq_scratch = yp if on_pe else xpb2
            for hh in range(nch):
                lo = hh * (NPIX // nch)
                hi = lo + NPIX // nch
                if first_batch:
                    nc.vector.scalar_tensor_tensor(
                        out=sq_scratch[:, lo:hi], in0=xff[:, lo:hi], scalar=1.0,
                        in1=xff[:, lo:hi], op0=ALU.mult, op1=ALU.mult,
                        accum_out=sumsq[:, hh : hh + 1],
                    )
                else:
                    nc.scalar.activation(
                        out=sq_scratch[:, lo:hi], in_=xff[:, lo:hi],
                        func=AF.Square, accum_out=sumsq[:, hh : hh + 1],
                    )
            # margins stay finite (reads run into them)
            nc.vector.memset(xpb[:, 0:MARG], 0.0)
            nc.vector.memset(xpb[:, FLAT - 2 : FLAT], 0.0)
            if not on_pe:
                nc.vector.memset(xpb2[:, FLAT - 2 : FLAT], 0.0)
            # Per chunk: f32->bf16 copy (+sum(x) accum) and reflect border
            # fills for the finished rows (all on ACT -- single-engine
            # chain, no cross-engine semaphore ping-pong).
            gx = grid(xpb[:])
            for hh in range(nch):
                nc.scalar.activation(
                    out=gx[:, 1 + hh * rows_c : 1 + (hh + 1) * rows_c, 1 : 1 + W],
                    in_=xf[:, hh * rows_c : (hh + 1) * rows_c],
                    func=AF.Copy, accum_out=sumx[:, hh : hh + 1],
                )
                lo_r, hi_r = 1 + hh * rows_c, (hh + 1) * rows_c
                nc.scalar.copy(out=gx[:, lo_r : hi_r + 1, 0:1], in_=gx[:, lo_r : hi_r + 1, 2:3])
                nc.scalar.copy(out=gx[:, lo_r : hi_r + 1, 65:66], in_=gx[:, lo_r : hi_r + 1, 63:64])
                if hh == 0:
                    nc.scalar.copy(out=gx[:, 0], in_=gx[:, 2])
                if hh == nch - 1:
                    nc.scalar.copy(out=gx[:, HPAD - 1], in_=gx[:, HPAD - 3])
            if not on_pe:
                # shifted copy (one element left) for 4B-aligned odd-offset
                # taps -- ONE op: each ACT copy carries ~1us fixed config
                # overhead so chunking this is a net loss.  NOTE: gpsimd
                # bulk ops are poison here -- they hold the shared
                # DVE/GpSimd SBUF port and stall every DVE tensor_tensor op.
                nc.scalar.copy(out=xpb2[:, 0 : FLAT - 2], in_=xpb[:, 1 : FLAT - 1])

            # ---- stats finalize (tiny per-partition ops) ----
            sx = small_pool.tile([128, 1], f32, tag="sx")
            nc.vector.reduce_sum(sx[:], sumx[:], axis=mybir.AxisListType.X)
            sq = small_pool.tile([128, 1], f32, tag="sq")
            nc.vector.reduce_sum(sq[:], sumsq[:], axis=mybir.AxisListType.X)
            mu = small_pool.tile([128, 1], f32, tag="mu")
            nc.vector.tensor_scalar_mul(mu[:], sx[:], 1.0 / NPIX)
            m2 = small_pool.tile([128, 1], f32, tag="m2")
            nc.vector.tensor_scalar_mul(m2[:], sq[:], 1.0 / NPIX)
            musq = small_pool.tile([128, 1], f32, tag="musq")
            nc.vector.tensor_mul(musq[:], mu[:], mu[:])
            var = small_pool.tile([128, 1], f32, tag="var")
            nc.vector.tensor_sub(var[:], m2[:], musq[:])
            std = small_pool.tile([128, 1], f32, tag="std")
            nc.scalar.activation(out=std[:], in_=var[:], func=AF.Sqrt, bias=eps_sb[:])
            rstd = small_pool.tile([128, 1], f32, tag="rstd")
            nc.vector.reciprocal(out=rstd[:], in_=std[:])
            a_sc = small_pool.tile([128, 1], f32, tag="a_sc")
            nc.vector.tensor_mul(a_sc[:], rstd[:], wpt[:])
            sw = small_pool.tile([128, 1], f32, tag="sw")
            nc.vector.reduce_sum(sw[:], wsp[:], axis=mybir.AxisListType.X)
            t1 = small_pool.tile([128, 1], f32, tag="t1")
            nc.vector.tensor_mul(t1[:], mu[:], a_sc[:])
            nc.vector.tensor_mul(t1[:], t1[:], sw[:])
            bconst = small_pool.tile([128, 1], f32, tag="bconst")
            nc.vector.tensor_sub(bconst[:], bis[:], t1[:])
            wsc = small_pool.tile([128, 9], f32, tag="wsc")
            nc.vector.tensor_scalar_mul(wsc[:], wsp[:], a_sc[:])
            if on_pe:
                # blip dummies paced by the stats chain keep the HAM MID
                # window from seeing a fully-idle 3.4us before the dw starts
                warm_mm(sq[:], 1, f"{b}_{ct}_bq")
                warm_mm(wsc[:], 9, f"{b}_{ct}_bw")

            # ---- depthwise: 9 taps over one flat 1D range ----
            # out positions [68, 4292) cover the whole interior; junk lands
            # on pad positions and is overwritten by the border fills.
            OUT0, OLEN = 68, 4224
            taps = [(dh, dw) for dh in (-1, 0, 1) for dw in (-1, 0, 1)]
            yp_seg = yp[:, OUT0 : OUT0 + OLEN]

            def tap_src(dh, dw):
                t_off = WPAD * dh + dw
                if dw == 0:
                    s = OUT0 + t_off       # even
                    return xpb[:, s : s + OLEN]
                s = OUT0 - 1 + t_off       # even (t_off odd)
                return xpb2[:, s : s + OLEN]

            if on_pe:
                # 9 diagonal-lhsT matmuls per 512-pixel segment, accumulated
                # in PSUM; ACT drains psum + B into yp (identity w/ bias).
                # Border fills happen INCREMENTALLY on DVE as segments
                # drain, so the big conv's first matmul waits only ~1us
                # after the last drain instead of on 4 queued ACT copies.
                diagw = small_pool.tile([128, 9, 128], bf16, tag="diagw")
                for t in range(9):
                    nc.vector.tensor_scalar_mul(
                        diagw[:, t, :], ident[:], wsc[:, t : t + 1])
                nc.vector.memset(yp[:, 0:MARG], 0.0)
                nc.vector.memset(yp[:, FLAT - 2 : FLAT], 0.0)
                seg_info = []
                s = OUT0
                while s < OUT0 + OLEN:
                    n = min(512, OUT0 + OLEN - s)
                    ps = psum_pool.tile([128, 512], f32, tag="ps",
                                        name=f"psdw_{b}_{ct}_{s}")
                    for t, (dh, dw) in enumerate(taps):
                        toff = WPAD * dh + dw
                        nc.tensor.matmul(
                            ps[:, :n], diagw[:, t, :], xpb[:, s + toff : s + toff + n],
                            start=(t == 0), stop=(t == 8),
                        )
                    if not seg_info:
                        # drain seg 1 inline: seg 9 reuses its PSUM ring
                        # slot, so its reader must be emitted before seg
                        # 9's allocation
                        nc.scalar.activation(
                            out=yp[:, s : s + n], in_=ps[:, :n],
                            func=AF.Identity, bias=bconst[:],
                        )
                    seg_info.append((ps, s, n))
                    s += n
                yp_tiles[(b, ct)] = yp

                def post_drains():
                    # drains 2-9 + reflect border fills are deadline-
                    # tolerant (each has ~15us of slack until its ct0
                    # bank's matmuls); emitting them AFTER b0ct1's
                    # copy/shift chain gives the batch-0 critical path the
                    # ACT queue first
                    g = grid(yp[:])
                    last_row = 0
                    for k, (ps, sk, nk) in enumerate(seg_info):
                        if k > 0:
                            nc.scalar.activation(
                                out=yp[:, sk : sk + nk], in_=ps[:, :nk],
                                func=AF.Identity, bias=bconst[:],
                            )
                        hi = min((sk + nk - OUT0) // WPAD, HPAD - 2)
                        if hi >= last_row + 1:
                            lo = last_row + 1
                            nc.scalar.copy(
                                out=g[:, lo : hi + 1, 0:1], in_=g[:, lo : hi + 1, 2:3])
                            nc.scalar.copy(
                                out=g[:, lo : hi + 1, 65:66], in_=g[:, lo : hi + 1, 63:64])
                            if last_row == 0:  # top reflect row (needs row 2)
                                nc.scalar.copy(out=g[:, 0], in_=g[:, 2])
                            last_row = hi
                    nc.scalar.copy(out=g[:, HPAD - 1], in_=g[:, HPAD - 3])

                return [post_drains]

            # Two tap products run on ACT (it has slack; saves 2 DVE
            # tensor_scalars per tile so DVE/batch stays well under the
            # PE's 61us).  Each ACT product is emitted as two half-length
            # ops so urgent ACT work (psum drains) is never stuck behind a
            # 4us copy.  DVE does the other 6 products (4x-mode
            # tensor_scalar) + the init tap and an 8-add pairwise tree
            # arranged so the ACT products are consumed last.  The init
            # tap and the first DVE product read the UNSHIFTED buffer so
            # the DVE chain starts before the ACT shift finishes; the
            # (0,0) ACT product does too.
            # b0ct1: 3 products on ACT (shortest possible DVE chain on the
            # batch-0 critical path); steady state: 2 (ACT/batch stays
            # under the PE's 61us)
            if first_batch:
                act_taps = [(0, 0), (1, 1), (1, -1)]
                rest = [(1, 0), (-1, -1), (-1, 1), (0, -1), (0, 1)]
            else:
                act_taps = [(0, 0), (1, 1)]
                rest = [(1, 0), (-1, -1), (-1, 1), (0, -1), (0, 1), (1, -1)]

            def emit_act_products():
                bufs = []
                for i, tp in enumerate(act_taps):
                    buf = tmp_pool.tile([128, OLEN], bf16, tag=f"dwtmp{i}",
                                        bufs=1, name=f"dwt{i}_{b}_{ct}")
                    t = (tp[0] + 1) * 3 + (tp[1] + 1)
                    nc.scalar.mul(buf[:], tap_src(*tp), wsc[:, t : t + 1])
                    bufs.append(buf)
                return bufs

            if not first_batch:
                tmps = emit_act_products()

            # init tap (-1,0) (even offset) writes yp_seg = w*x + B directly
            d0, w0 = (-1, 0)
            t0 = (d0 + 1) * 3 + (w0 + 1)
            nc.vector.tensor_scalar(
                yp_seg, tap_src(d0, w0), wsc[:, t0 : t0 + 1], bconst[:],
                op0=ALU.mult, op1=ALU.add,
            )

            def mul_into(buf, tap):
                t = (tap[0] + 1) * 3 + (tap[1] + 1)
                nc.vector.tensor_scalar_mul(buf[:], tap_src(*tap), wsc[:, t : t + 1])

            ta = tmp_pool.tile([128, OLEN], bf16, tag="dwA", bufs=1, name=f"dwA_{b}_{ct}")
            tb = tmp_pool.tile([128, OLEN], bf16, tag="dwB", bufs=1, name=f"dwB_{b}_{ct}")
            tc_ = tmp_pool.tile([128, OLEN], bf16, tag="dwC", bufs=1, name=f"dwC_{b}_{ct}")
            mul_into(ta, rest[0])
            mul_into(tb, rest[1])
            nc.vector.tensor_add(ta[:], ta[:], tb[:])
            mul_into(tb, rest[2])
            mul_into(tc_, rest[3])
            nc.vector.tensor_add(tb[:], tb[:], tc_[:])
            nc.vector.tensor_add(ta[:], ta[:], tb[:])        # DVE taps 1-4
            mul_into(tb, rest[4])
            if len(rest) == 6:
                mul_into(tc_, rest[5])
                nc.vector.tensor_add(tb[:], tb[:], tc_[:])   # DVE taps 5-6
            nc.vector.tensor_add(yp_seg, yp_seg, ta[:])
            nc.vector.memset(yp[:, 0:MARG], 0.0)
            nc.vector.memset(yp[:, FLAT - 2 : FLAT], 0.0)
            yp_tiles[(b, ct)] = yp

            def finish(tmps):
                nc.vector.tensor_add(tc_[:], tmps[0][:], tmps[1][:])  # ACT taps
                nc.vector.tensor_add(tb[:], tb[:], tc_[:])
                if len(tmps) == 3:
                    nc.vector.tensor_add(tb[:], tb[:], tmps[2][:])
                nc.vector.tensor_add(yp_seg, yp_seg, tb[:])
                fill_borders_dve(yp)

            if first_batch:
                # for b0ct1, products + tail adds are emitted after ct0's
                # deferred drains so the ACT queue order matches deadlines
                def post_products():
                    finish(emit_act_products())
                return [post_products]
            finish(tmps)
            return []

        def big_conv(b):
            # ot -> ct -> bank(r0) -> tap: all 8 banks held; 72-matmul
            # runway on ct0 before yp[ct1] is needed; each bank drains on
            # ACT (identity + conv bias) and DMAs its 8-row block out as
            # soon as its ct1 taps finish.
            for ot in range(OT):
                stage = stage_pool.tile([128, H, W], f32, tag="stage")
                ps = {}
                for r0, nr in ROW_BLOCKS:
                    ps[r0] = psum_pool.tile(
                        [128, BLK_ROWS * W], f32, tag="ps",
                        name=f"ps_{b}_{ot}_{r0}",
                    )
                for ct in range(CT):
                    ypg = grid(yp_tiles[(b, ct)][:])
                    for r0, nr in ROW_BLOCKS:
                        i = 0
                        for dh in (-1, 0, 1):
                            for dw in (-1, 0, 1):
                                kh, kw = dh + 1, dw + 1
                                lhsT = cw_sb[ct][:, kh, kw, ot, :]
                                rhs = ypg[:, r0 + 1 + dh : r0 + 1 + dh + nr,
                                          1 + dw : 1 + dw + W]
                                nc.tensor.matmul(
                                    ps[r0][:], lhsT, rhs,
                                    start=(ct == 0 and i == 0),
                                    stop=(ct == CT - 1 and i == 8),
                                )
                                i += 1
                        if ct == CT - 1:
                            src = ps[r0][:].rearrange("p (r c) -> p r c", c=W)
                            nc.scalar.activation(
                                out=stage[:, r0 : r0 + nr, :], in_=src,
                                func=AF.Identity, bias=cb_sb[:, ot : ot + 1],
                            )
                            nc.sync.dma_start(
                                out=out_ext[b, ot * 128 : (ot + 1) * 128,
                                            r0 : r0 + nr],
                                in_=stage[:, r0 : r0 + nr],
                            )

        for b in range(nb):
            xf_ct1 = None
            if b == 0:
                # load b0ct1's x over the ACTIVATION engine's DMA queue
                # (ACT is idle this early, and its hardware queue runs
                # concurrently with the sync queue carrying ct0) so ct1
                # lands ~19us instead of serializing behind ct0's 2MB at
                # ~27us
                xf_ct1 = xin_pool.tile([128, H, W], f32, tag="xf")
                rc = H // 4
                for q in range(4):
                    nc.scalar.dma_start(
                        out=xf_ct1[:, q * rc : (q + 1) * rc],
                        in_=x_ext[b, 128:256, q * rc : (q + 1) * rc],
                    )
            posts = []
            for ct in range(CT):
                posts += produce_yp(b, ct, xf_pre=xf_ct1 if ct == 1 else None) or []
            for p in posts:   # b0: ct0's deferred drains, then ct1's products
                p()
            if b == 0:
                load_cw()  # after batch 0's x DMAs are queued
            big_conv(b)

    nc.compile()
    return nc


def _host_prep(x, w_spatial, w_pointwise, bias, conv_w, conv_b, nb=NB):
    import ml_dtypes

    ncores = x.shape[0] // nb
    cw = np.ascontiguousarray(
        conv_w.reshape(OT, 128, CT, 128, 3, 3).transpose(2, 3, 4, 5, 0, 1)
    ).astype(ml_dtypes.bfloat16)
    cb = np.ascontiguousarray(conv_b.reshape(OT, 128)).astype(np.float32)
    wsp = np.ascontiguousarray(w_spatial.reshape(-1, CT, 128, 9)).astype(np.float32)
    wpt = np.ascontiguousarray(w_pointwise.reshape(-1, CT, 128)).astype(np.float32)
    bis = np.ascontiguousarray(bias.reshape(-1, CT, 128)).astype(np.float32)
    x = np.ascontiguousarray(x).astype(np.float32)
    in_maps = []
    for i in range(ncores):
        sl = slice(i * nb, (i + 1) * nb)
        in_maps.append({
            "x": np.ascontiguousarray(x[sl]),
            "wsp": np.ascontiguousarray(wsp[sl]),
            "wpt": np.ascontiguousarray(wpt[sl]),
            "bis": np.ascontiguousarray(bis[sl]),
            "cw": cw,
            "cb": cb,
        })
    return in_maps


def _run(inputs, trace=False):
    from concourse.bass_utils import run_bass_kernel_spmd

    if "nc" not in _CACHED:
        _CACHED["nc"] = _build()
    nc = _CACHED["nc"]
    in_maps = _host_prep(**inputs)
    kw = {}
    if trace:
        import shutil
        tdir = "/tmp/kernel_trace_out"
        shutil.rmtree(tdir, ignore_errors=True)
        os.makedirs(tdir, exist_ok=True)
        kw["tmpdir"] = tdir
    res = run_bass_kernel_spmd(
        nc, in_maps, core_ids=list(range(N_CORES)), trace=trace, **kw
    )
    out = np.concatenate([res.results[i]["out"] for i in range(N_CORES)], axis=0)
    return out.astype(np.float32), res


def kernel(x, w_spatial, w_pointwise, bias, conv_w, conv_b):
    out, _ = _run(
        dict(x=np.asarray(x), w_spatial=np.asarray(w_spatial),
             w_pointwise=np.asarray(w_pointwise), bias=np.asarray(bias),
             conv_w=np.asarray(conv_w), conv_b=np.asarray(conv_b)),
        trace=bool(int(os.environ.get("KERNEL_TRACE", "0"))),
    )
    return out


# revision 29
# speedup vs baseline: 1.0347x; 1.0347x over previous
"""AdaConv2d fused kernel for 8 TRN2 NeuronCores (pure data parallel).

Per-sample pipeline (all fused on-chip):
  1. instance-norm stats (mean/var over HW)
  2. dynamic per-(b,c) depthwise 3x3 conv with reflect padding
  3. per-(b,c) scale+bias (folded algebraically into the depthwise taps:
     y = A*(sum_t w_t * x_t) + B with A = rstd*w_pt, B = bias - mu*A*sum(w))
  4. fixed 3x3 conv (256->256) with reflect padding, as 18 accumulated
     bf16 matmuls per PSUM bank

Layout: channels on partitions (2 tiles of 128), pixels on the free axis.
Padded images are 66 rows x 66 cols stored flat with a 2-element leading
margin (so every depthwise tap and every matmul rhs is a fully CONTIGUOUS
1D slice).  flat(r, c) = 2 + 66*r + c.  Rows 0/65 and cols 0/65 are the
reflect pads.  A one-element-left-shifted copy (xpb2[i] = xpb[i+1]) keeps
all odd-offset depthwise taps 4-byte aligned for the DVE bf16 2x mode.

v2 scheduling (engine budget per core: PE 263us, DVE ~200us, ACT ~205us):
  - batch 0 tiles use quarter-chunked x DMAs + stats so the pipeline
    latency to the first PE matmul shrinks; sum(x^2) for b0 tiles runs on
    DVE (scalar_tensor_tensor w/ accum) because DVE is idle at startup.
  - dummy fp32 matmuls paced by the b0 DMA chunk arrivals keep the PE HAM
    clock-gate warm so the b0ct0 depthwise-on-PE runs at 2.4 GHz.
  - b0ct0 depthwise runs on the PE (diagonal matmuls) so the PE is not
    gated on DVE at startup; its PSUM drains go to ACT (identity+bias).
  - per tile, 2 of the 8 non-center tap products run on ACT; DVE does
    6 products + 7 tree adds (DVE/batch ~49us vs PE/batch 61us).
  - big conv: ot -> ct -> bank(r0) -> tap order for every batch: 8 PSUM
    banks held, 72-MM runway before ct1 is needed, banks drain (ACT) and
    DMA out per 8-row block as soon as their ct1 taps finish.
"""

import os
from contextlib import ExitStack

import numpy as np

B_GLOBAL = 32
N_CORES = 8
NB = B_GLOBAL // N_CORES  # batches per core
C = 256
H = W = 64
WPAD = W + 2        # 66 padded row length
HPAD = H + 2        # 66 padded rows
MARG = 2            # leading margin so tap windows stay in-bounds
FLAT = MARG + HPAD * WPAD + 2   # 4360 flat elements per padded image
NPIX = H * W        # 4096
CT = C // 128       # channel tiles
OT = C // 128       # out-channel tiles
EPS = 1e-5
BLK_ROWS = 8        # output rows per PSUM bank (8*64=512 fp32, 3D-AP rhs)

ROW_BLOCKS = [(r0, BLK_ROWS) for r0 in range(0, H, BLK_ROWS)]

_CACHED = {}


def _build(nb=NB):
    import concourse.mybir as mybir
    import concourse.tile as tile
    from concourse import bacc

    f32 = mybir.dt.float32
    bf16 = mybir.dt.bfloat16
    AF = mybir.ActivationFunctionType
    ALU = mybir.AluOpType

    nc = bacc.Bacc(None, target_bir_lowering=False)

    x_ext = nc.declare_dram_parameter("x", [nb, C, H, W], f32, isOutput=False)
    wsp_ext = nc.declare_dram_parameter("wsp", [nb, CT, 128, 9], f32, isOutput=False)
    wpt_ext = nc.declare_dram_parameter("wpt", [nb, CT, 128], f32, isOutput=False)
    bis_ext = nc.declare_dram_parameter("bis", [nb, CT, 128], f32, isOutput=False)
    cw_ext = nc.declare_dram_parameter("cw", [CT, 128, 3, 3, OT, 128], bf16, isOutput=False)
    cb_ext = nc.declare_dram_parameter("cb", [OT, 128], f32, isOutput=False)
    out_ext = nc.declare_dram_parameter("out", [nb, C, H, W], f32, isOutput=True)

    with tile.TileContext(nc) as tc, ExitStack() as ctx:
        singles = ctx.enter_context(tc.tile_pool(name="singles", bufs=1))
        xin_pool = ctx.enter_context(tc.tile_pool(name="xin", bufs=2))
        xpb_pool = ctx.enter_context(tc.tile_pool(name="xpb", bufs=2))
        xpb2_pool = ctx.enter_context(tc.tile_pool(name="xpb2", bufs=2))
        yp_pool = ctx.enter_context(tc.tile_pool(name="yp", bufs=4))
        stage_pool = ctx.enter_context(tc.tile_pool(name="stage", bufs=2))
        small_pool = ctx.enter_context(tc.tile_pool(name="small", bufs=4))
        tmp_pool = ctx.enter_context(tc.tile_pool(name="tmp", bufs=2))
        psum_pool = ctx.enter_context(tc.tile_pool(name="psum", bufs=8, space="PSUM"))

        # ---- constants / fixed weights ----
        # (emitted lazily AFTER batch 0's x DMAs so the 1.2MB weight load
        # doesn't delay the first tile's data; weights are only needed at
        # ~30us when the first big-conv matmul fires)
        cw_sb = []

        cb_sb = singles.tile([128, OT], f32, tag="cb")

        def load_cw():
            for ct in range(CT):
                t = singles.tile([128, 3, 3, OT, 128], bf16, tag=f"cw{ct}")
                nc.sync.dma_start(out=t[:], in_=cw_ext[ct])
                cw_sb.append(t)
            for ot in range(OT):
                nc.sync.dma_start(out=cb_sb[:, ot : ot + 1], in_=cb_ext[ot, :, None])

        eps_sb = singles.tile([128, 1], f32, tag="eps")
        nc.vector.memset(eps_sb[:], EPS)
        ident = singles.tile([128, 128], bf16, tag="ident")
        from concourse.masks import make_identity
        make_identity(nc, ident[:])
        # junk fp32 lhsT for the HAM warm-up matmuls
        junkw = singles.tile([128, 128], f32, tag="junkw")
        nc.vector.memset(junkw[:], 0.5)
        # touch the Sqrt activation table once so its ~1.3us load happens
        # before the first tile's stats need it
        warm = singles.tile([128, 1], f32, tag="warm")
        nc.scalar.activation(out=warm[:], in_=eps_sb[:], func=AF.Sqrt, bias=eps_sb[:])

        yp_tiles = {}

        def grid(flat_ap):
            """(128, FLAT) flat padded buffer -> (128, 66, 66) image view."""
            return flat_ap[:, MARG : MARG + HPAD * WPAD].rearrange(
                "p (r c) -> p r c", c=WPAD)

        def fill_borders_dve(buf):
            g = grid(buf[:])
            nc.vector.tensor_scalar_mul(g[:, 1 : 1 + H, 0:1], g[:, 1 : 1 + H, 2:3], 1.0)
            nc.vector.tensor_scalar_mul(g[:, 1 : 1 + H, 65:66], g[:, 1 : 1 + H, 63:64], 1.0)
            nc.vector.tensor_scalar_mul(g[:, 0], g[:, 2], 1.0)
            nc.vector.tensor_scalar_mul(g[:, HPAD - 1], g[:, HPAD - 3], 1.0)

        def warm_mm(rhs_f32, n, label, count=1):
            """Dummy fp32 matmuls that keep the PE HAM clock-gate warm
            while real work is still in flight. Results are never read."""
            ps = psum_pool.tile([128, 512], f32, tag="ps", name=f"psw_{label}")
            for i in range(count):
                nc.tensor.matmul(ps[:, :n], junkw[:], rhs_f32, start=True, stop=True)

        def produce_yp(b, ct, xf_pre=None):
            """norm + depthwise pipeline for one (batch, channel-tile)."""
            first_batch = b == 0
            on_pe = first_batch and ct == 0
            nch = 4 if first_batch else 2       # x DMA / stats chunks
            rows_c = H // nch

            if xf_pre is not None:
                xf = xf_pre
            else:
                xf = xin_pool.tile([128, H, W], f32, tag="xf")
            # fp32 dummies run at HALF rate (427ns warm / 853 cold per
            # N=512) so keep the warm-up lean: enough sustained busy to
            # trip the HAM SHORT window (~3.4us), then stop.
            warm_counts = {0: 3, 1: 2, 2: 1, 3: 1}
            for q in range(nch):
                if xf_pre is None:
                    nc.sync.dma_start(
                        out=xf[:, q * rows_c : (q + 1) * rows_c],
                        in_=x_ext[b, ct * 128 : (ct + 1) * 128, q * rows_c : (q + 1) * rows_c],
                    )
                if on_pe:
                    warm_mm(
                        xf[:, q * rows_c : (q + 1) * rows_c]
                        .rearrange("p a b -> p (a b)")[:, :512],
                        512, f"{b}_{ct}_{q}", warm_counts[q],
                    )

            wsp = small_pool.tile([128, 9], f32, tag="wsp")
            nc.sync.dma_start(out=wsp[:], in_=wsp_ext[b, ct])
            wpt = small_pool.tile([128, 1], f32, tag="wpt")
            nc.sync.dma_start(out=wpt[:], in_=wpt_ext[b, ct, :, None])
            bis = small_pool.tile([128, 1], f32, tag="bis")
            nc.sync.dma_start(out=bis[:], in_=bis_ext[b, ct, :, None])

            xpb = xpb_pool.tile([128, FLAT], bf16, tag="xpb")
            xpb2 = xpb2_pool.tile([128, FLAT], bf16, tag="xpb2")

            # stats: sum(x) fused into the f32->bf16 convert on ACT.
            # sum(x^2): batch-0 tiles compute it on DVE (idle at startup)
            # via scalar_tensor_tensor w/ accum; later tiles use ACT Square.
            # Square scratch: b0ct0 -> yp buffer (overwritten by PE-dw
            # drains later); other tiles -> xpb2 (overwritten by the shift).
            sumx = small_pool.tile([128, nch], f32, tag="sumx")
            sumsq = small_pool.tile([128, nch], f32, tag="sumsq")
            xff = xf[:].rearrange("p a b -> p (a b)")
            yp = yp_pool.tile([128, FLAT], bf16, tag="yp")
            sq_scratch = yp if on_pe else xpb2
            for hh in range(nch):
                lo = hh * (NPIX // nch)
                hi = lo + NPIX // nch
                if first_batch:
                    nc.vector.scalar_tensor_tensor(
                        out=sq_scratch[:, lo:hi], in0=xff[:, lo:hi], scalar=1.0,
                        in1=xff[:, lo:hi], op0=ALU.mult, op1=ALU.mult,
                        accum_out=sumsq[:, hh : hh + 1],
                    )
                else:
                    nc.scalar.activation(
                        out=sq_scratch[:, lo:hi], in_=xff[:, lo:hi],
                        func=AF.Square, accum_out=sumsq[:, hh : hh + 1],
                    )
            # margins stay finite (reads run into them)
            nc.vector.memset(xpb[:, 0:MARG], 0.0)
            nc.vector.memset(xpb[:, FLAT - 2 : FLAT], 0.0)
            if not on_pe:
                nc.vector.memset(xpb2[:, FLAT - 2 : FLAT], 0.0)
            # Per chunk: f32->bf16 copy (+sum(x) accum) and reflect border
            # fills for the finished rows (all on ACT -- single-engine
            # chain, no cross-engine semaphore ping-pong).
            gx = grid(xpb[:])
            for hh in range(nch):
                nc.scalar.activation(
                    out=gx[:, 1 + hh * rows_c : 1 + (hh + 1) * rows_c, 1 : 1 + W],
                    in_=xf[:, hh * rows_c : (hh + 1) * rows_c],
                    func=AF.Copy, accum_out=sumx[:, hh : hh + 1],
                )
                lo_r, hi_r = 1 + hh * rows_c, (hh + 1) * rows_c
                nc.scalar.copy(out=gx[:, lo_r : hi_r + 1, 0:1], in_=gx[:, lo_r : hi_r + 1, 2:3])
                nc.scalar.copy(out=gx[:, lo_r : hi_r + 1, 65:66], in_=gx[:, lo_r : hi_r + 1, 63:64])
                if hh == 0:
                    nc.scalar.copy(out=gx[:, 0], in_=gx[:, 2])
                if hh == nch - 1:
                    nc.scalar.copy(out=gx[:, HPAD - 1], in_=gx[:, HPAD - 3])
            if not on_pe:
                # shifted copy (one element left) for 4B-aligned odd-offset
                # taps -- ONE op: each ACT copy carries ~1us fixed config
                # overhead so chunking this is a net loss.  NOTE: gpsimd
                # bulk ops are poison here -- they hold the shared
                # DVE/GpSimd SBUF port and stall every DVE tensor_tensor op.
                nc.scalar.copy(out=xpb2[:, 0 : FLAT - 2], in_=xpb[:, 1 : FLAT - 1])

            # ---- stats finalize (tiny per-partition ops) ----
            sx = small_pool.tile([128, 1], f32, tag="sx")
            nc.vector.reduce_sum(sx[:], sumx[:], axis=mybir.AxisListType.X)
            sq = small_pool.tile([128, 1], f32, tag="sq")
            nc.vector.reduce_sum(sq[:], sumsq[:], axis=mybir.AxisListType.X)
            mu = small_pool.tile([128, 1], f32, tag="mu")
            nc.vector.tensor_scalar_mul(mu[:], sx[:], 1.0 / NPIX)
            m2 = small_pool.tile([128, 1], f32, tag="m2")
            nc.vector.tensor_scalar_mul(m2[:], sq[:], 1.0 / NPIX)
            musq = small_pool.tile([128, 1], f32, tag="musq")
            nc.vector.tensor_mul(musq[:], mu[:], mu[:])
            var = small_pool.tile([128, 1], f32, tag="var")
            nc.vector.tensor_sub(var[:], m2[:], musq[:])
            std = small_pool.tile([128, 1], f32, tag="std")
            nc.scalar.activation(out=std[:], in_=var[:], func=AF.Sqrt, bias=eps_sb[:])
            rstd = small_pool.tile([128, 1], f32, tag="rstd")
            nc.vector.reciprocal(out=rstd[:], in_=std[:])
            a_sc = small_pool.tile([128, 1], f32, tag="a_sc")
            nc.vector.tensor_mul(a_sc[:], rstd[:], wpt[:])
            sw = small_pool.tile([128, 1], f32, tag="sw")
            nc.vector.reduce_sum(sw[:], wsp[:], axis=mybir.AxisListType.X)
            t1 = small_pool.tile([128, 1], f32, tag="t1")
            nc.vector.tensor_mul(t1[:], mu[:], a_sc[:])
            nc.vector.tensor_mul(t1[:], t1[:], sw[:])
            bconst = small_pool.tile([128, 1], f32, tag="bconst")
            nc.vector.tensor_sub(bconst[:], bis[:], t1[:])
            wsc = small_pool.tile([128, 9], f32, tag="wsc")
            nc.vector.tensor_scalar_mul(wsc[:], wsp[:], a_sc[:])
            if on_pe:
                # blip dummies paced by the stats chain keep the HAM MID
                # window from seeing a fully-idle 3.4us before the dw starts
                warm_mm(sq[:], 1, f"{b}_{ct}_bq")
                warm_mm(wsc[:], 9, f"{b}_{ct}_bw")

            # ---- depthwise: 9 taps over one flat 1D range ----
            # out positions [68, 4292) cover the whole interior; junk lands
            # on pad positions and is overwritten by the border fills.
            OUT0, OLEN = 68, 4224
            taps = [(dh, dw) for dh in (-1, 0, 1) for dw in (-1, 0, 1)]
            yp_seg = yp[:, OUT0 : OUT0 + OLEN]

            def tap_src(dh, dw):
                t_off = WPAD * dh + dw
                if dw == 0:
                    s = OUT0 + t_off       # even
                    return xpb[:, s : s + OLEN]
                s = OUT0 - 1 + t_off       # even (t_off odd)
                return xpb2[:, s : s + OLEN]

            if on_pe:
                # 9 diagonal-lhsT matmuls per 512-pixel segment, accumulated
                # in PSUM; ACT drains psum + B into yp (identity w/ bias).
                # Border fills happen INCREMENTALLY on DVE as segments
                # drain, so the big conv's first matmul waits only ~1us
                # after the last drain instead of on 4 queued ACT copies.
                diagw = small_pool.tile([128, 9, 128], bf16, tag="diagw")
                for t in range(9):
                    nc.vector.tensor_scalar_mul(
                        diagw[:, t, :], ident[:], wsc[:, t : t + 1])
                nc.vector.memset(yp[:, 0:MARG], 0.0)
                nc.vector.memset(yp[:, FLAT - 2 : FLAT], 0.0)
                seg_info = []
                s = OUT0
                while s < OUT0 + OLEN:
                    n = min(512, OUT0 + OLEN - s)
                    ps = psum_pool.tile([128, 512], f32, tag="ps",
                                        name=f"psdw_{b}_{ct}_{s}")
                    for t, (dh, dw) in enumerate(taps):
                        toff = WPAD * dh + dw
                        nc.tensor.matmul(
                            ps[:, :n], diagw[:, t, :], xpb[:, s + toff : s + toff + n],
                            start=(t == 0), stop=(t == 8),
                        )
                    if not seg_info:
                        # drain seg 1 inline: seg 9 reuses its PSUM ring
                        # slot, so its reader must be emitted before seg
                        # 9's allocation
                        nc.scalar.activation(
                            out=yp[:, s : s + n], in_=ps[:, :n],
                            func=AF.Identity, bias=bconst[:],
                        )
                    seg_info.append((ps, s, n))
                    s += n
                yp_tiles[(b, ct)] = yp

                def post_drains():
                    # drains 2-9 + reflect border fills are deadline-
                    # tolerant (each has ~15us of slack until its ct0
                    # bank's matmuls); emitting them AFTER b0ct1's
                    # copy/shift chain gives the batch-0 critical path the
                    # ACT queue first
                    g = grid(yp[:])
                    last_row = 0
                    for k, (ps, sk, nk) in enumerate(seg_info):
                        if k > 0:
                            nc.scalar.activation(
                                out=yp[:, sk : sk + nk], in_=ps[:, :nk],
                                func=AF.Identity, bias=bconst[:],
                            )
                        hi = min((sk + nk - OUT0) // WPAD, HPAD - 2)
                        if hi >= last_row + 1:
                            lo = last_row + 1
                            nc.scalar.copy(
                                out=g[:, lo : hi + 1, 0:1], in_=g[:, lo : hi + 1, 2:3])
                            nc.scalar.copy(
                                out=g[:, lo : hi + 1, 65:66], in_=g[:, lo : hi + 1, 63:64])
                            if last_row == 0:  # top reflect row (needs row 2)
                                nc.scalar.copy(out=g[:, 0], in_=g[:, 2])
                            last_row = hi
                    nc.scalar.copy(out=g[:, HPAD - 1], in_=g[:, HPAD - 3])

                return [post_drains]

            # Two tap products run on ACT (it has slack; saves 2 DVE
            # tensor_scalars per tile so DVE/batch stays well under the
            # PE's 61us).  Each ACT product is emitted as two half-length
            # ops so urgent ACT work (psum drains) is never stuck behind a
            # 4us copy.  DVE does the other 6 products (4x-mode
            # tensor_scalar) + the init tap and an 8-add pairwise tree
            # arranged so the ACT products are consumed last.  The init
            # tap and the first DVE product read the UNSHIFTED buffer so
            # the DVE chain starts before the ACT shift finishes; the
            # (0,0) ACT product does too.
            # b0ct1: 3 products on ACT (shortest possible DVE chain on the
            # batch-0 critical path); steady state: 2 (ACT/batch stays
            # under the PE's 61us)
            if first_batch:
                act_taps = [(0, 0), (1, 1), (1, -1)]
                rest = [(1, 0), (-1, -1), (-1, 1), (0, -1), (0, 1)]
            else:
                act_taps = [(0, 0), (1, 1)]
                rest = [(1, 0), (-1, -1), (-1, 1), (0, -1), (0, 1), (1, -1)]

            def emit_act_products():
                bufs = []
                for i, tp in enumerate(act_taps):
                    buf = tmp_pool.tile([128, OLEN], bf16, tag=f"dwtmp{i}",
                                        bufs=1, name=f"dwt{i}_{b}_{ct}")
                    t = (tp[0] + 1) * 3 + (tp[1] + 1)
                    nc.scalar.mul(buf[:], tap_src(*tp), wsc[:, t : t + 1])
                    bufs.append(buf)
                return bufs

            if not first_batch:
                tmps = emit_act_products()

            # init tap (-1,0) (even offset) writes yp_seg = w*x + B directly
            d0, w0 = (-1, 0)
            t0 = (d0 + 1) * 3 + (w0 + 1)
            nc.vector.tensor_scalar(
                yp_seg, tap_src(d0, w0), wsc[:, t0 : t0 + 1], bconst[:],
                op0=ALU.mult, op1=ALU.add,
            )

            def mul_into(buf, tap):
                t = (tap[0] + 1) * 3 + (tap[1] + 1)
                nc.vector.tensor_scalar_mul(buf[:], tap_src(*tap), wsc[:, t : t + 1])

            ta = tmp_pool.tile([128, OLEN], bf16, tag="dwA", bufs=1, name=f"dwA_{b}_{ct}")
            tb = tmp_pool.tile([128, OLEN], bf16, tag="dwB", bufs=1, name=f"dwB_{b}_{ct}")
            tc_ = tmp_pool.tile([128, OLEN], bf16, tag="dwC", bufs=1, name=f"dwC_{b}_{ct}")
            mul_into(ta, rest[0])
            mul_into(tb, rest[1])
            nc.vector.tensor_add(ta[:], ta[:], tb[:])
            mul_into(tb, rest[2])
            mul_into(tc_, rest[3])
            nc.vector.tensor_add(tb[:], tb[:], tc_[:])
            nc.vector.tensor_add(ta[:], ta[:], tb[:])        # DVE taps 1-4
            mul_into(tb, rest[4])
            if len(rest) == 6:
                mul_into(tc_, rest[5])
                nc.vector.tensor_add(tb[:], tb[:], tc_[:])   # DVE taps 5-6
            nc.vector.tensor_add(yp_seg, yp_seg, ta[:])
            nc.vector.memset(yp[:, 0:MARG], 0.0)
            nc.vector.memset(yp[:, FLAT - 2 : FLAT], 0.0)
            yp_tiles[(b, ct)] = yp

            def finish(tmps):
                nc.vector.tensor_add(tc_[:], tmps[0][:], tmps[1][:])  # ACT taps
                nc.vector.tensor_add(tb[:], tb[:], tc_[:])
                if len(tmps) == 3:
                    nc.vector.tensor_add(tb[:], tb[:], tmps[2][:])
                nc.vector.tensor_add(yp_seg, yp_seg, tb[:])
                fill_borders_dve(yp)

            if first_batch:
                # for b0ct1, products + tail adds are emitted after ct0's
                # deferred drains so the ACT queue order matches deadlines
                def post_products():
                    finish(emit_act_products())
                return [post_products]
            finish(tmps)
            return []

        def big_conv(b):
            # ot -> ct -> bank(r0) -> tap: all 8 banks held; 72-matmul
            # runway on ct0 before yp[ct1] is needed; each bank drains on
            # ACT (identity + conv bias) and DMAs its 8-row block out as
            # soon as its ct1 taps finish.
            for ot in range(OT):
                stage = stage_pool.tile([128, H, W], f32, tag="stage")
                ps = {}
                for r0, nr in ROW_BLOCKS:
                    ps[r0] = psum_pool.tile(
                        [128, BLK_ROWS * W], f32, tag="ps",
                        name=f"ps_{b}_{ot}_{r0}",
                    )
                for ct in range(CT):
                    ypg = grid(yp_tiles[(b, ct)][:])
                    for r0, nr in ROW_BLOCKS:
                        i = 0
                        for dh in (-1, 0, 1):
                            for dw in (-1, 0, 1):
                                kh, kw = dh + 1, dw + 1
                                lhsT = cw_sb[ct][:, kh, kw, ot, :]
                                rhs = ypg[:, r0 + 1 + dh : r0 + 1 + dh + nr,
                                          1 + dw : 1 + dw + W]
                                nc.tensor.matmul(
                                    ps[r0][:], lhsT, rhs,
                                    start=(ct == 0 and i == 0),
                                    stop=(ct == CT - 1 and i == 8),
                                )
                                i += 1
                        if ct == CT - 1:
                            src = ps[r0][:].rearrange("p (r c) -> p r c", c=W)
                            nc.scalar.activation(
                                out=stage[:, r0 : r0 + nr, :], in_=src,
                                func=AF.Identity, bias=cb_sb[:, ot : ot + 1],
                            )
                            nc.sync.dma_start(
                                out=out_ext[b, ot * 128 : (ot + 1) * 128,
                                            r0 : r0 + nr],
                                in_=stage[:, r0 : r0 + nr],
                            )

        for b in range(nb):
            posts = []
            for ct in range(CT):
                posts += produce_yp(b, ct) or []
            for p in posts:   # b0: ct0's deferred drains, then ct1's products
                p()
            if b == 0:
                load_cw()  # after batch 0's x DMAs are queued
            big_conv(b)

    nc.compile()
    return nc


def _host_prep(x, w_spatial, w_pointwise, bias, conv_w, conv_b, nb=NB):
    import ml_dtypes

    ncores = x.shape[0] // nb
    cw = np.ascontiguousarray(
        conv_w.reshape(OT, 128, CT, 128, 3, 3).transpose(2, 3, 4, 5, 0, 1)
    ).astype(ml_dtypes.bfloat16)
    cb = np.ascontiguousarray(conv_b.reshape(OT, 128)).astype(np.float32)
    wsp = np.ascontiguousarray(w_spatial.reshape(-1, CT, 128, 9)).astype(np.float32)
    wpt = np.ascontiguousarray(w_pointwise.reshape(-1, CT, 128)).astype(np.float32)
    bis = np.ascontiguousarray(bias.reshape(-1, CT, 128)).astype(np.float32)
    x = np.ascontiguousarray(x).astype(np.float32)
    in_maps = []
    for i in range(ncores):
        sl = slice(i * nb, (i + 1) * nb)
        in_maps.append({
            "x": np.ascontiguousarray(x[sl]),
            "wsp": np.ascontiguousarray(wsp[sl]),
            "wpt": np.ascontiguousarray(wpt[sl]),
            "bis": np.ascontiguousarray(bis[sl]),
            "cw": cw,
            "cb": cb,
        })
    return in_maps


def _run(inputs, trace=False):
    from concourse.bass_utils import run_bass_kernel_spmd

    if "nc" not in _CACHED:
        _CACHED["nc"] = _build()
    nc = _CACHED["nc"]
    in_maps = _host_prep(**inputs)
    kw = {}
    if trace:
        import shutil
        tdir = "/tmp/kernel_trace_out"
        shutil.rmtree(tdir, ignore_errors=True)
        os.makedirs(tdir, exist_ok=True)
        kw["tmpdir"] = tdir
    try:
        res = run_bass_kernel_spmd(
            nc, in_maps, core_ids=list(range(N_CORES)), trace=trace, **kw
        )
        out = np.concatenate([res.results[i]["out"] for i in range(N_CORES)], axis=0)
    except Exception:
        # one retry: a previously wedged device surfaces as a transient
        # NRT_EXEC_UNIT_UNRECOVERABLE on the first execution after it
        res = run_bass_kernel_spmd(
            nc, in_maps, core_ids=list(range(N_CORES)), trace=trace, **kw
        )
        out = np.concatenate([res.results[i]["out"] for i in range(N_CORES)], axis=0)
    return out.astype(np.float32), res


def kernel(x, w_spatial, w_pointwise, bias, conv_w, conv_b):
    out, _ = _run(
        dict(x=np.asarray(x), w_spatial=np.asarray(w_spatial),
             w_pointwise=np.asarray(w_pointwise), bias=np.asarray(bias),
             conv_w=np.asarray(conv_w), conv_b=np.asarray(conv_b)),
        trace=bool(int(os.environ.get("KERNEL_TRACE", "0"))),
    )
    return out


# revision 30
# speedup vs baseline: 1.0624x; 1.0268x over previous
"""AdaConv2d fused kernel for 8 TRN2 NeuronCores (pure data parallel).

Per-sample pipeline (all fused on-chip):
  1. instance-norm stats (mean/var over HW)
  2. dynamic per-(b,c) depthwise 3x3 conv with reflect padding
  3. per-(b,c) scale+bias (folded algebraically into the depthwise taps:
     y = A*(sum_t w_t * x_t) + B with A = rstd*w_pt, B = bias - mu*A*sum(w))
  4. fixed 3x3 conv (256->256) with reflect padding, as 18 accumulated
     bf16 matmuls per PSUM bank

Layout: channels on partitions (2 tiles of 128), pixels on the free axis.
Padded images are 66 rows x 66 cols stored flat with a 2-element leading
margin (so every depthwise tap and every matmul rhs is a fully CONTIGUOUS
1D slice).  flat(r, c) = 2 + 66*r + c.  Rows 0/65 and cols 0/65 are the
reflect pads.  A one-element-left-shifted copy (xpb2[i] = xpb[i+1]) keeps
all odd-offset depthwise taps 4-byte aligned for the DVE bf16 2x mode.

v2 scheduling (engine budget per core: PE 263us, DVE ~200us, ACT ~205us):
  - batch 0 tiles use quarter-chunked x DMAs + stats so the pipeline
    latency to the first PE matmul shrinks; sum(x^2) for b0 tiles runs on
    DVE (scalar_tensor_tensor w/ accum) because DVE is idle at startup.
  - dummy fp32 matmuls paced by the b0 DMA chunk arrivals keep the PE HAM
    clock-gate warm so the b0ct0 depthwise-on-PE runs at 2.4 GHz.
  - b0ct0 depthwise runs on the PE (diagonal matmuls) so the PE is not
    gated on DVE at startup; its PSUM drains go to ACT (identity+bias).
  - per tile, 2 of the 8 non-center tap products run on ACT; DVE does
    6 products + 7 tree adds (DVE/batch ~49us vs PE/batch 61us).
  - big conv: ot -> ct -> bank(r0) -> tap order for every batch: 8 PSUM
    banks held, 72-MM runway before ct1 is needed, banks drain (ACT) and
    DMA out per 8-row block as soon as their ct1 taps finish.
"""

import os
from contextlib import ExitStack

import numpy as np

B_GLOBAL = 32
N_CORES = 8
NB = B_GLOBAL // N_CORES  # batches per core
C = 256
H = W = 64
WPAD = W + 2        # 66 padded row length
HPAD = H + 2        # 66 padded rows
MARG = 2            # leading margin so tap windows stay in-bounds
FLAT = MARG + HPAD * WPAD + 2   # 4360 flat elements per padded image
NPIX = H * W        # 4096
CT = C // 128       # channel tiles
OT = C // 128       # out-channel tiles
EPS = 1e-5
BLK_ROWS = 8        # output rows per PSUM bank (8*64=512 fp32, 3D-AP rhs)

ROW_BLOCKS = [(r0, BLK_ROWS) for r0 in range(0, H, BLK_ROWS)]

_CACHED = {}


def _build(nb=NB):
    import concourse.mybir as mybir
    import concourse.tile as tile
    from concourse import bacc

    f32 = mybir.dt.float32
    bf16 = mybir.dt.bfloat16
    AF = mybir.ActivationFunctionType
    ALU = mybir.AluOpType

    nc = bacc.Bacc(None, target_bir_lowering=False)

    x_ext = nc.declare_dram_parameter("x", [nb, C, H, W], f32, isOutput=False)
    wsp_ext = nc.declare_dram_parameter("wsp", [nb, CT, 128, 9], f32, isOutput=False)
    wpt_ext = nc.declare_dram_parameter("wpt", [nb, CT, 128], f32, isOutput=False)
    bis_ext = nc.declare_dram_parameter("bis", [nb, CT, 128], f32, isOutput=False)
    cw_ext = nc.declare_dram_parameter("cw", [CT, 128, 3, 3, OT, 128], bf16, isOutput=False)
    cb_ext = nc.declare_dram_parameter("cb", [OT, 128], f32, isOutput=False)
    out_ext = nc.declare_dram_parameter("out", [nb, C, H, W], f32, isOutput=True)

    with tile.TileContext(nc) as tc, ExitStack() as ctx:
        singles = ctx.enter_context(tc.tile_pool(name="singles", bufs=1))
        xin_pool = ctx.enter_context(tc.tile_pool(name="xin", bufs=2))
        xpb_pool = ctx.enter_context(tc.tile_pool(name="xpb", bufs=2))
        xpb2_pool = ctx.enter_context(tc.tile_pool(name="xpb2", bufs=2))
        yp_pool = ctx.enter_context(tc.tile_pool(name="yp", bufs=4))
        stage_pool = ctx.enter_context(tc.tile_pool(name="stage", bufs=2))
        small_pool = ctx.enter_context(tc.tile_pool(name="small", bufs=4))
        tmp_pool = ctx.enter_context(tc.tile_pool(name="tmp", bufs=2))
        psum_pool = ctx.enter_context(tc.tile_pool(name="psum", bufs=8, space="PSUM"))

        # ---- constants / fixed weights ----
        # (emitted lazily AFTER batch 0's x DMAs so the 1.2MB weight load
        # doesn't delay the first tile's data; weights are only needed at
        # ~30us when the first big-conv matmul fires)
        cw_sb = []

        cb_sb = singles.tile([128, OT], f32, tag="cb")

        def load_cw():
            for ct in range(CT):
                t = singles.tile([128, 3, 3, OT, 128], bf16, tag=f"cw{ct}")
                nc.sync.dma_start(out=t[:], in_=cw_ext[ct])
                cw_sb.append(t)
            for ot in range(OT):
                nc.sync.dma_start(out=cb_sb[:, ot : ot + 1], in_=cb_ext[ot, :, None])

        eps_sb = singles.tile([128, 1], f32, tag="eps")
        nc.vector.memset(eps_sb[:], EPS)
        ident = singles.tile([128, 128], bf16, tag="ident")
        from concourse.masks import make_identity
        make_identity(nc, ident[:])
        # junk fp32 lhsT for the HAM warm-up matmuls
        junkw = singles.tile([128, 128], f32, tag="junkw")
        nc.vector.memset(junkw[:], 0.5)
        # touch the Sqrt activation table once so its ~1.3us load happens
        # before the first tile's stats need it
        warm = singles.tile([128, 1], f32, tag="warm")
        nc.scalar.activation(out=warm[:], in_=eps_sb[:], func=AF.Sqrt, bias=eps_sb[:])

        yp_tiles = {}

        def grid(flat_ap):
            """(128, FLAT) flat padded buffer -> (128, 66, 66) image view."""
            return flat_ap[:, MARG : MARG + HPAD * WPAD].rearrange(
                "p (r c) -> p r c", c=WPAD)

        def fill_borders_dve(buf):
            g = grid(buf[:])
            nc.vector.tensor_scalar_mul(g[:, 1 : 1 + H, 0:1], g[:, 1 : 1 + H, 2:3], 1.0)
            nc.vector.tensor_scalar_mul(g[:, 1 : 1 + H, 65:66], g[:, 1 : 1 + H, 63:64], 1.0)
            nc.vector.tensor_scalar_mul(g[:, 0], g[:, 2], 1.0)
            nc.vector.tensor_scalar_mul(g[:, HPAD - 1], g[:, HPAD - 3], 1.0)

        def warm_mm(rhs_f32, n, label, count=1):
            """Dummy fp32 matmuls that keep the PE HAM clock-gate warm
            while real work is still in flight. Results are never read."""
            ps = psum_pool.tile([128, 512], f32, tag="ps", name=f"psw_{label}")
            for i in range(count):
                nc.tensor.matmul(ps[:, :n], junkw[:], rhs_f32, start=True, stop=True)

        def produce_yp(b, ct):
            """norm + depthwise pipeline for one (batch, channel-tile)."""
            first_batch = b == 0
            on_pe = first_batch and ct == 0
            nch = 4 if first_batch else 2       # x DMA / stats chunks
            rows_c = H // nch

            xf = xin_pool.tile([128, H, W], f32, tag="xf")
            # fp32 dummies run at HALF rate (427ns warm / 853 cold per
            # N=512) so keep the warm-up lean: enough sustained busy to
            # trip the HAM SHORT window (~3.4us), then stop.
            warm_counts = {0: 3, 1: 2, 2: 1, 3: 1}
            for q in range(nch):
                nc.sync.dma_start(
                    out=xf[:, q * rows_c : (q + 1) * rows_c],
                    in_=x_ext[b, ct * 128 : (ct + 1) * 128, q * rows_c : (q + 1) * rows_c],
                )
                if on_pe:
                    warm_mm(
                        xf[:, q * rows_c : (q + 1) * rows_c]
                        .rearrange("p a b -> p (a b)")[:, :512],
                        512, f"{b}_{ct}_{q}", warm_counts[q],
                    )

            wsp = small_pool.tile([128, 9], f32, tag="wsp")
            nc.sync.dma_start(out=wsp[:], in_=wsp_ext[b, ct])
            wpt = small_pool.tile([128, 1], f32, tag="wpt")
            nc.sync.dma_start(out=wpt[:], in_=wpt_ext[b, ct, :, None])
            bis = small_pool.tile([128, 1], f32, tag="bis")
            nc.sync.dma_start(out=bis[:], in_=bis_ext[b, ct, :, None])

            xpb = xpb_pool.tile([128, FLAT], bf16, tag="xpb")
            xpb2 = xpb2_pool.tile([128, FLAT], bf16, tag="xpb2")

            # stats: sum(x) fused into the f32->bf16 convert on ACT.
            # sum(x^2): batch-0 tiles compute it on DVE (idle at startup)
            # via scalar_tensor_tensor w/ accum; later tiles use ACT Square.
            # Square scratch: b0ct0 -> yp buffer (overwritten by PE-dw
            # drains later); other tiles -> xpb2 (overwritten by the shift).
            sumx = small_pool.tile([128, nch], f32, tag="sumx")
            sumsq = small_pool.tile([128, nch], f32, tag="sumsq")
            xff = xf[:].rearrange("p a b -> p (a b)")
            yp = yp_pool.tile([128, FLAT], bf16, tag="yp")
            sq_scratch = yp if on_pe else xpb2
            for hh in range(nch):
                lo = hh * (NPIX // nch)
                hi = lo + NPIX // nch
                if first_batch:
                    nc.vector.scalar_tensor_tensor(
                        out=sq_scratch[:, lo:hi], in0=xff[:, lo:hi], scalar=1.0,
                        in1=xff[:, lo:hi], op0=ALU.mult, op1=ALU.mult,
                        accum_out=sumsq[:, hh : hh + 1],
                    )
                else:
                    nc.scalar.activation(
                        out=sq_scratch[:, lo:hi], in_=xff[:, lo:hi],
                        func=AF.Square, accum_out=sumsq[:, hh : hh + 1],
                    )
            # margins stay finite (reads run into them)
            nc.vector.memset(xpb[:, 0:MARG], 0.0)
            nc.vector.memset(xpb[:, FLAT - 2 : FLAT], 0.0)
            if not on_pe:
                nc.vector.memset(xpb2[:, FLAT - 2 : FLAT], 0.0)
            # Per chunk: f32->bf16 copy (+sum(x) accum) and reflect border
            # fills for the finished rows (all on ACT -- single-engine
            # chain, no cross-engine semaphore ping-pong).
            gx = grid(xpb[:])
            for hh in range(nch):
                nc.scalar.activation(
                    out=gx[:, 1 + hh * rows_c : 1 + (hh + 1) * rows_c, 1 : 1 + W],
                    in_=xf[:, hh * rows_c : (hh + 1) * rows_c],
                    func=AF.Copy, accum_out=sumx[:, hh : hh + 1],
                )
                lo_r, hi_r = 1 + hh * rows_c, (hh + 1) * rows_c
                nc.scalar.copy(out=gx[:, lo_r : hi_r + 1, 0:1], in_=gx[:, lo_r : hi_r + 1, 2:3])
                nc.scalar.copy(out=gx[:, lo_r : hi_r + 1, 65:66], in_=gx[:, lo_r : hi_r + 1, 63:64])
                if hh == 0:
                    nc.scalar.copy(out=gx[:, 0], in_=gx[:, 2])
                if hh == nch - 1:
                    nc.scalar.copy(out=gx[:, HPAD - 1], in_=gx[:, HPAD - 3])
            if not on_pe:
                # shifted copy (one element left) for 4B-aligned odd-offset
                # taps -- ONE op: each ACT copy carries ~1us fixed config
                # overhead so chunking this is a net loss.  NOTE: gpsimd
                # bulk ops are poison here -- they hold the shared
                # DVE/GpSimd SBUF port and stall every DVE tensor_tensor op.
                nc.scalar.copy(out=xpb2[:, 0 : FLAT - 2], in_=xpb[:, 1 : FLAT - 1])

            # ---- stats finalize (tiny per-partition ops) ----
            sx = small_pool.tile([128, 1], f32, tag="sx")
            nc.vector.reduce_sum(sx[:], sumx[:], axis=mybir.AxisListType.X)
            sq = small_pool.tile([128, 1], f32, tag="sq")
            nc.vector.reduce_sum(sq[:], sumsq[:], axis=mybir.AxisListType.X)
            mu = small_pool.tile([128, 1], f32, tag="mu")
            nc.vector.tensor_scalar_mul(mu[:], sx[:], 1.0 / NPIX)
            m2 = small_pool.tile([128, 1], f32, tag="m2")
            nc.vector.tensor_scalar_mul(m2[:], sq[:], 1.0 / NPIX)
            musq = small_pool.tile([128, 1], f32, tag="musq")
            nc.vector.tensor_mul(musq[:], mu[:], mu[:])
            var = small_pool.tile([128, 1], f32, tag="var")
            nc.vector.tensor_sub(var[:], m2[:], musq[:])
            std = small_pool.tile([128, 1], f32, tag="std")
            nc.scalar.activation(out=std[:], in_=var[:], func=AF.Sqrt, bias=eps_sb[:])
            rstd = small_pool.tile([128, 1], f32, tag="rstd")
            nc.vector.reciprocal(out=rstd[:], in_=std[:])
            a_sc = small_pool.tile([128, 1], f32, tag="a_sc")
            nc.vector.tensor_mul(a_sc[:], rstd[:], wpt[:])
            sw = small_pool.tile([128, 1], f32, tag="sw")
            nc.vector.reduce_sum(sw[:], wsp[:], axis=mybir.AxisListType.X)
            t1 = small_pool.tile([128, 1], f32, tag="t1")
            nc.vector.tensor_mul(t1[:], mu[:], a_sc[:])
            nc.vector.tensor_mul(t1[:], t1[:], sw[:])
            bconst = small_pool.tile([128, 1], f32, tag="bconst")
            nc.vector.tensor_sub(bconst[:], bis[:], t1[:])
            wsc = small_pool.tile([128, 9], f32, tag="wsc")
            nc.vector.tensor_scalar_mul(wsc[:], wsp[:], a_sc[:])
            if on_pe:
                # blip dummies paced by the stats chain keep the HAM MID
                # window from seeing a fully-idle 3.4us before the dw starts
                warm_mm(sq[:], 1, f"{b}_{ct}_bq")
                warm_mm(wsc[:], 9, f"{b}_{ct}_bw")

            # ---- depthwise: 9 taps over one flat 1D range ----
            # out positions [68, 4292) cover the whole interior; junk lands
            # on pad positions and is overwritten by the border fills.
            OUT0, OLEN = 68, 4224
            taps = [(dh, dw) for dh in (-1, 0, 1) for dw in (-1, 0, 1)]
            yp_seg = yp[:, OUT0 : OUT0 + OLEN]

            def tap_src(dh, dw):
                t_off = WPAD * dh + dw
                if dw == 0:
                    s = OUT0 + t_off       # even
                    return xpb[:, s : s + OLEN]
                s = OUT0 - 1 + t_off       # even (t_off odd)
                return xpb2[:, s : s + OLEN]

            if on_pe:
                # 9 diagonal-lhsT matmuls per 512-pixel segment, accumulated
                # in PSUM; ACT drains psum + B into yp (identity w/ bias).
                # Border fills happen INCREMENTALLY on DVE as segments
                # drain, so the big conv's first matmul waits only ~1us
                # after the last drain instead of on 4 queued ACT copies.
                diagw = small_pool.tile([128, 9, 128], bf16, tag="diagw")
                for t in range(9):
                    nc.vector.tensor_scalar_mul(
                        diagw[:, t, :], ident[:], wsc[:, t : t + 1])
                nc.vector.memset(yp[:, 0:MARG], 0.0)
                nc.vector.memset(yp[:, FLAT - 2 : FLAT], 0.0)
                seg_info = []
                s = OUT0
                while s < OUT0 + OLEN:
                    n = min(512, OUT0 + OLEN - s)
                    ps = psum_pool.tile([128, 512], f32, tag="ps",
                                        name=f"psdw_{b}_{ct}_{s}")
                    for t, (dh, dw) in enumerate(taps):
                        toff = WPAD * dh + dw
                        nc.tensor.matmul(
                            ps[:, :n], diagw[:, t, :], xpb[:, s + toff : s + toff + n],
                            start=(t == 0), stop=(t == 8),
                        )
                    if not seg_info:
                        # drain seg 1 inline: seg 9 reuses its PSUM ring
                        # slot, so its reader must be emitted before seg
                        # 9's allocation
                        nc.scalar.activation(
                            out=yp[:, s : s + n], in_=ps[:, :n],
                            func=AF.Identity, bias=bconst[:],
                        )
                    seg_info.append((ps, s, n))
                    s += n
                yp_tiles[(b, ct)] = yp

                def post_drains():
                    # drains 2-9 + reflect border fills are deadline-
                    # tolerant (each has ~15us of slack until its ct0
                    # bank's matmuls); emitting them AFTER b0ct1's
                    # copy/shift chain gives the batch-0 critical path the
                    # ACT queue first
                    g = grid(yp[:])
                    last_row = 0
                    for k, (ps, sk, nk) in enumerate(seg_info):
                        if k > 0:
                            nc.scalar.activation(
                                out=yp[:, sk : sk + nk], in_=ps[:, :nk],
                                func=AF.Identity, bias=bconst[:],
                            )
                        hi = min((sk + nk - OUT0) // WPAD, HPAD - 2)
                        if hi >= last_row + 1:
                            lo = last_row + 1
                            nc.scalar.copy(
                                out=g[:, lo : hi + 1, 0:1], in_=g[:, lo : hi + 1, 2:3])
                            nc.scalar.copy(
                                out=g[:, lo : hi + 1, 65:66], in_=g[:, lo : hi + 1, 63:64])
                            if last_row == 0:  # top reflect row (needs row 2)
                                nc.scalar.copy(out=g[:, 0], in_=g[:, 2])
                            last_row = hi
                    nc.scalar.copy(out=g[:, HPAD - 1], in_=g[:, HPAD - 3])

                return [post_drains]

            # Two tap products run on ACT (it has slack; saves 2 DVE
            # tensor_scalars per tile so DVE/batch stays well under the
            # PE's 61us).  Each ACT product is emitted as two half-length
            # ops so urgent ACT work (psum drains) is never stuck behind a
            # 4us copy.  DVE does the other 6 products (4x-mode
            # tensor_scalar) + the init tap and an 8-add pairwise tree
            # arranged so the ACT products are consumed last.  The init
            # tap and the first DVE product read the UNSHIFTED buffer so
            # the DVE chain starts before the ACT shift finishes; the
            # (0,0) ACT product does too.
            act_taps = [(0, 0), (1, 1)]
            HLEN = OLEN // 2

            def emit_act_products():
                tmpa = tmp_pool.tile([128, OLEN], bf16, tag="dwtmpa", bufs=1, name=f"dwta_{b}_{ct}")
                tmpb = tmp_pool.tile([128, OLEN], bf16, tag="dwtmpb", bufs=1, name=f"dwtb_{b}_{ct}")
                for buf, tp in zip((tmpa, tmpb), act_taps):
                    t = (tp[0] + 1) * 3 + (tp[1] + 1)
                    src = tap_src(*tp)
                    nc.scalar.mul(buf[:, :HLEN], src[:, :HLEN], wsc[:, t : t + 1])
                    nc.scalar.mul(buf[:, HLEN:], src[:, HLEN:], wsc[:, t : t + 1])
                return tmpa, tmpb

            if not first_batch:
                tmpab = emit_act_products()

            # init tap (-1,0) (even offset) writes yp_seg = w*x + B directly
            d0, w0 = (-1, 0)
            t0 = (d0 + 1) * 3 + (w0 + 1)
            nc.vector.tensor_scalar(
                yp_seg, tap_src(d0, w0), wsc[:, t0 : t0 + 1], bconst[:],
                op0=ALU.mult, op1=ALU.add,
            )
            # DVE taps (6): (1,0) is even (unshifted) and runs first
            rest = [(1, 0), (-1, -1), (-1, 1), (0, -1), (0, 1), (1, -1)]

            def mul_into(buf, tap):
                t = (tap[0] + 1) * 3 + (tap[1] + 1)
                nc.vector.tensor_scalar_mul(buf[:], tap_src(*tap), wsc[:, t : t + 1])

            ta = tmp_pool.tile([128, OLEN], bf16, tag="dwA", bufs=1, name=f"dwA_{b}_{ct}")
            tb = tmp_pool.tile([128, OLEN], bf16, tag="dwB", bufs=1, name=f"dwB_{b}_{ct}")
            tc_ = tmp_pool.tile([128, OLEN], bf16, tag="dwC", bufs=1, name=f"dwC_{b}_{ct}")
            mul_into(ta, rest[0])
            mul_into(tb, rest[1])
            nc.vector.tensor_add(ta[:], ta[:], tb[:])
            mul_into(tb, rest[2])
            mul_into(tc_, rest[3])
            nc.vector.tensor_add(tb[:], tb[:], tc_[:])
            nc.vector.tensor_add(ta[:], ta[:], tb[:])        # DVE taps 1-4
            mul_into(tb, rest[4])
            mul_into(tc_, rest[5])
            nc.vector.tensor_add(tb[:], tb[:], tc_[:])       # DVE taps 5-6
            nc.vector.tensor_add(yp_seg, yp_seg, ta[:])
            nc.vector.memset(yp[:, 0:MARG], 0.0)
            nc.vector.memset(yp[:, FLAT - 2 : FLAT], 0.0)
            yp_tiles[(b, ct)] = yp

            def finish(tmpa, tmpb):
                nc.vector.tensor_add(tc_[:], tmpa[:], tmpb[:])   # ACT taps
                nc.vector.tensor_add(tb[:], tb[:], tc_[:])
                nc.vector.tensor_add(yp_seg, yp_seg, tb[:])
                fill_borders_dve(yp)

            if first_batch:
                # for b0ct1, products + tail adds are emitted after ct0's
                # deferred drains so the ACT queue order matches deadlines
                def post_products():
                    finish(*emit_act_products())
                return [post_products]
            finish(*tmpab)
            return []

        def big_conv(b):
            # ot -> ct -> bank(r0) -> tap: all 8 banks held; 72-matmul
            # runway on ct0 before yp[ct1] is needed; each bank drains on
            # ACT (identity + conv bias) and DMAs its 8-row block out as
            # soon as its ct1 taps finish.
            for ot in range(OT):
                stage = stage_pool.tile([128, H, W], f32, tag="stage")
                ps = {}
                for r0, nr in ROW_BLOCKS:
                    ps[r0] = psum_pool.tile(
                        [128, BLK_ROWS * W], f32, tag="ps",
                        name=f"ps_{b}_{ot}_{r0}",
                    )
                for ct in range(CT):
                    ypg = grid(yp_tiles[(b, ct)][:])
                    for r0, nr in ROW_BLOCKS:
                        i = 0
                        for dh in (-1, 0, 1):
                            for dw in (-1, 0, 1):
                                kh, kw = dh + 1, dw + 1
                                lhsT = cw_sb[ct][:, kh, kw, ot, :]
                                rhs = ypg[:, r0 + 1 + dh : r0 + 1 + dh + nr,
                                          1 + dw : 1 + dw + W]
                                nc.tensor.matmul(
                                    ps[r0][:], lhsT, rhs,
                                    start=(ct == 0 and i == 0),
                                    stop=(ct == CT - 1 and i == 8),
                                )
                                i += 1
                        if ct == CT - 1:
                            src = ps[r0][:].rearrange("p (r c) -> p r c", c=W)
                            nc.scalar.activation(
                                out=stage[:, r0 : r0 + nr, :], in_=src,
                                func=AF.Identity, bias=cb_sb[:, ot : ot + 1],
                            )
                            nc.sync.dma_start(
                                out=out_ext[b, ot * 128 : (ot + 1) * 128,
                                            r0 : r0 + nr],
                                in_=stage[:, r0 : r0 + nr],
                            )

        for b in range(nb):
            posts = []
            for ct in range(CT):
                posts += produce_yp(b, ct) or []
            for p in posts:   # b0: ct0's deferred drains, then ct1's products
                p()
            if b == 0:
                load_cw()  # after batch 0's x DMAs are queued
            big_conv(b)

    nc.compile()
    return nc


def _host_prep(x, w_spatial, w_pointwise, bias, conv_w, conv_b, nb=NB):
    import ml_dtypes

    ncores = x.shape[0] // nb
    cw = np.ascontiguousarray(
        conv_w.reshape(OT, 128, CT, 128, 3, 3).transpose(2, 3, 4, 5, 0, 1)
    ).astype(ml_dtypes.bfloat16)
    cb = np.ascontiguousarray(conv_b.reshape(OT, 128)).astype(np.float32)
    wsp = np.ascontiguousarray(w_spatial.reshape(-1, CT, 128, 9)).astype(np.float32)
    wpt = np.ascontiguousarray(w_pointwise.reshape(-1, CT, 128)).astype(np.float32)
    bis = np.ascontiguousarray(bias.reshape(-1, CT, 128)).astype(np.float32)
    x = np.ascontiguousarray(x).astype(np.float32)
    in_maps = []
    for i in range(ncores):
        sl = slice(i * nb, (i + 1) * nb)
        in_maps.append({
            "x": np.ascontiguousarray(x[sl]),
            "wsp": np.ascontiguousarray(wsp[sl]),
            "wpt": np.ascontiguousarray(wpt[sl]),
            "bis": np.ascontiguousarray(bis[sl]),
            "cw": cw,
            "cb": cb,
        })
    return in_maps


def _run(inputs, trace=False):
    from concourse.bass_utils import run_bass_kernel_spmd

    if "nc" not in _CACHED:
        _CACHED["nc"] = _build()
    nc = _CACHED["nc"]
    in_maps = _host_prep(**inputs)
    kw = {}
    if trace:
        import shutil
        tdir = "/tmp/kernel_trace_out"
        shutil.rmtree(tdir, ignore_errors=True)
        os.makedirs(tdir, exist_ok=True)
        kw["tmpdir"] = tdir
    try:
        res = run_bass_kernel_spmd(
            nc, in_maps, core_ids=list(range(N_CORES)), trace=trace, **kw
        )
        out = np.concatenate([res.results[i]["out"] for i in range(N_CORES)], axis=0)
    except Exception:
        # one retry: a previously wedged device surfaces as a transient
        # NRT_EXEC_UNIT_UNRECOVERABLE on the first execution after it
        res = run_bass_kernel_spmd(
            nc, in_maps, core_ids=list(range(N_CORES)), trace=trace, **kw
        )
        out = np.concatenate([res.results[i]["out"] for i in range(N_CORES)], axis=0)
    return out.astype(np.float32), res


def kernel(x, w_spatial, w_pointwise, bias, conv_w, conv_b):
    out, _ = _run(
        dict(x=np.asarray(x), w_spatial=np.asarray(w_spatial),
             w_pointwise=np.asarray(w_pointwise), bias=np.asarray(bias),
             conv_w=np.asarray(conv_w), conv_b=np.asarray(conv_b)),
        trace=bool(int(os.environ.get("KERNEL_TRACE", "0"))),
    )
    return out
